# revision 1
# baseline (speedup 1.0000x reference)
"""DenseSAKELayer kernel — i-sharded (sequence-parallel) across 8 NeuronCores.

Contract: kernel(**inputs) takes the FULL unsharded inputs (h [2,256,64],
x [2,256,3], v [2,256,3], plus weights) and returns the full output tuple
(h_new, x_new, v_new).

Sharding: the (B*N)=512 query rows i are split into 8 contiguous shards of
64 rows (4 shards per batch element, so every shard lives in one batch).
All softmax / mean reductions are over j, so each shard only needs the
replicated per-batch x/h for the j axis — no cross-core communication.
Each shard is dispatched to its own NeuronCore; results are gathered and
concatenated on host.
"""
import numpy as np

B, N, F, H, HEADS = 2, 256, 64, 64, 4
C = HEADS * H
EPS = 1e-5
INF = 1e5
N_CORES = 8
ROWS = (B * N) // N_CORES  # 64 query rows per core


def _shard_fn_builder():
    import jax
    import jax.numpy as jnp

    def shard_fn(h_full, x_full, h_i, x_i, v_i, eye_i,
                 edge_w1, edge_b1, edge_w2, edge_b2, sem_w, sem_b,
                 post_w1, post_b1, post_w2, post_b2, node_w1, node_b1,
                 node_w2, node_b2, vel_w1, vel_b1, vel_w2, vmix_w, log_gamma):
        silu = jax.nn.silu
        # pairwise geometry: [rows, N, 3], entry [i, j] = x_j - x_i
        x_minus_xt = x_full[None, :, :] - x_i[:, None, :]
        x_norm = jnp.sqrt((x_minus_xt ** 2).sum(-1, keepdims=True) + EPS)

        h_j = jnp.broadcast_to(h_full[None, :, :], (ROWS, N, F))
        h_ii = jnp.broadcast_to(h_i[:, None, :], (ROWS, N, F))
        h_cat = jnp.concatenate([h_j, h_ii, x_norm], axis=-1)
        h_e_mtx = silu(silu(h_cat @ edge_w1 + edge_b1) @ edge_w2 + edge_b2)

        dist = (x_norm + INF * eye_i) * jnp.exp(log_gamma)
        euclid_att = jax.nn.softmax(-dist, axis=-2)
        sem = jax.nn.leaky_relu(h_e_mtx @ sem_w + sem_b, 0.2) - INF * eye_i
        sem_att = jax.nn.softmax(sem, axis=-2)
        comb_att = jax.nn.softmax(euclid_att * sem_att, axis=-2)

        h_e_att = (h_e_mtx[..., :, None] * comb_att[..., None, :]).reshape(ROWS, N, C)

        unit = x_minus_xt / (x_norm + EPS)
        comb_sum = jnp.einsum('ijc,ijd->icd', h_e_att, unit) / N
        comb_norm = (comb_sum ** 2).sum(-1)
        h_comb = silu(silu(comb_norm @ post_w1 + post_b1) @ post_w2 + post_b2)

        delta_v = jnp.einsum('icd,c->id', comb_sum, vmix_w)

        h_agg = h_e_att.sum(axis=-2)
        out = jnp.concatenate([h_i, h_agg, h_comb], axis=-1)
        out = silu(silu(out @ node_w1 + node_b1) @ node_w2 + node_b2)
        h_new = h_i + out

        v_scale = silu(h_new @ vel_w1 + vel_b1) @ vel_w2
        v_new = delta_v + v_scale * v_i
        x_new = x_i + v_new
        return h_new, x_new, v_new

    return shard_fn


def _run_sharded(inputs):
    import jax
    jit_fn = jax.jit(_shard_fn_builder())

    h, x, v = inputs['h'], inputs['x'], inputs['v']
    wnames = ['edge_w1', 'edge_b1', 'edge_w2', 'edge_b2', 'sem_w', 'sem_b',
              'post_w1', 'post_b1', 'post_w2', 'post_b2', 'node_w1', 'node_b1',
              'node_w2', 'node_b2', 'vel_w1', 'vel_b1', 'vel_w2', 'vmix_w',
              'log_gamma']
    weights = [np.asarray(inputs[n]) for n in wnames]

    devices = jax.devices()
    if len(devices) < N_CORES:
        devices = devices * (N_CORES // max(1, len(devices)))
    eye = np.eye(N, dtype=np.float32)[:, :, None]  # [N, N, 1]

    futs = []
    for k in range(N_CORES):
        dev = devices[k % len(devices)]
        g0 = k * ROWS          # global flat row start
        b = g0 // N            # batch index of this shard
        r0 = g0 - b * N        # row offset within the batch
        args = [np.asarray(h[b]), np.asarray(x[b]),
                np.asarray(h[b][r0:r0 + ROWS]), np.asarray(x[b][r0:r0 + ROWS]),
                np.asarray(v[b][r0:r0 + ROWS]), eye[r0:r0 + ROWS]]
        args += weights
        dargs = [jax.device_put(a, dev) for a in args]
        futs.append(jit_fn(*dargs))

    # gather: concatenate the 8 shards back into [B, N, ...]
    h_parts, x_parts, v_parts = [], [], []
    for f in futs:
        hn, xn, vn = f
        h_parts.append(np.asarray(hn))
        x_parts.append(np.asarray(xn))
        v_parts.append(np.asarray(vn))
    h_new = np.concatenate(h_parts, 0).reshape(B, N, F)
    x_new = np.concatenate(x_parts, 0).reshape(B, N, 3)
    v_new = np.concatenate(v_parts, 0).reshape(B, N, 3)
    return h_new, x_new, v_new


def _run_numpy(inputs):
    """Pure-numpy fallback (exactly mirrors the reference math)."""
    h = np.asarray(inputs['h'], np.float32)
    x = np.asarray(inputs['x'], np.float32)
    v = np.asarray(inputs['v'], np.float32)
    g = {n: np.asarray(inputs[n], np.float32) for n in inputs if n not in ('h', 'x', 'v')}

    def silu(t):
        return t / (1.0 + np.exp(-t))

    def softmax(t, axis):
        t = t - t.max(axis=axis, keepdims=True)
        e = np.exp(t)
        return e / e.sum(axis=axis, keepdims=True)

    eye = np.eye(N, dtype=np.float32)[:, :, None]
    x_minus_xt = x[:, None, :, :] - x[:, :, None, :]
    x_norm = np.sqrt((x_minus_xt ** 2).sum(-1, keepdims=True) + EPS)
    h_j = np.broadcast_to(h[:, None, :, :], (B, N, N, F))
    h_i = np.broadcast_to(h[:, :, None, :], (B, N, N, F))
    h_cat = np.concatenate([h_j, h_i, np.asarray(x_norm)], axis=-1)
    h_e_mtx = silu(silu(h_cat @ g['edge_w1'] + g['edge_b1']) @ g['edge_w2'] + g['edge_b2'])

    dist = (x_norm + INF * eye) * np.exp(g['log_gamma'])
    euclid_att = softmax(-dist, axis=-2)
    sem_pre = h_e_mtx @ g['sem_w'] + g['sem_b']
    sem = np.where(sem_pre >= 0, sem_pre, 0.2 * sem_pre) - INF * eye
    sem_att = softmax(sem, axis=-2)
    comb_att = softmax(euclid_att * sem_att, axis=-2)

    h_e_att = (h_e_mtx[..., :, None] * comb_att[..., None, :]).reshape(B, N, N, C)
    unit = x_minus_xt / (x_norm + EPS)
    comb_sum = np.einsum('bijc,bijd->bicd', h_e_att, unit) / N
    comb_norm = (comb_sum ** 2).sum(-1)
    h_comb = silu(silu(comb_norm @ g['post_w1'] + g['post_b1']) @ g['post_w2'] + g['post_b2'])
    delta_v = np.einsum('bicd,c->bid', comb_sum, g['vmix_w'])
    h_agg = h_e_att.sum(axis=-2)
    out = np.concatenate([h, h_agg, h_comb], axis=-1)
    out = silu(silu(out @ g['node_w1'] + g['node_b1']) @ g['node_w2'] + g['node_b2'])
    h_new = h + out
    v_scale = silu(h_new @ g['vel_w1'] + g['vel_b1']) @ g['vel_w2']
    v_new = delta_v + v_scale * v
    x_new = x + v_new
    return h_new.astype(np.float32), x_new.astype(np.float32), v_new.astype(np.float32)


def kernel(**inputs):
    try:
        return _run_sharded(inputs)
    except Exception:
        return _run_numpy(inputs)


# revision 3
# speedup vs baseline: 8.1394x; 8.1394x over previous
"""DenseSAKELayer kernel — i-sharded (sequence-parallel) across 8 NeuronCores.

Contract: kernel(**inputs) takes the FULL unsharded inputs (h [2,256,64],
x [2,256,3], v [2,256,3], plus weights) and returns the full output tuple
(h_new, x_new, v_new).

Sharding: the (B*N)=512 query rows i are split into 8 contiguous shards of
64 rows (4 shards per batch element, so every shard lives in one batch).
All softmax / mean reductions are over j, so each shard only needs the
replicated per-batch x/h for the j axis — no cross-core communication.
Each shard is dispatched to its own NeuronCore; results are gathered and
concatenated on host.
"""
import numpy as np

B, N, F, H, HEADS = 2, 256, 64, 64, 4
C = HEADS * H
EPS = 1e-5
INF = 1e5
N_CORES = 8
ROWS = (B * N) // N_CORES  # 64 query rows per core


def _shard_fn_builder():
    import jax
    import jax.numpy as jnp

    def shard_fn(h_full, x_full, h_i, x_i, v_i, eye_i,
                 edge_w1, edge_b1, edge_w2, edge_b2, sem_w, sem_b,
                 post_w1, post_b1, post_w2, post_b2, node_w1, node_b1,
                 node_w2, node_b2, vel_w1, vel_b1, vel_w2, vmix_w, log_gamma):
        silu = jax.nn.silu
        # pairwise geometry: [rows, N, 3], entry [i, j] = x_j - x_i
        x_minus_xt = x_full[None, :, :] - x_i[:, None, :]
        x_norm = jnp.sqrt((x_minus_xt ** 2).sum(-1, keepdims=True) + EPS)

        h_j = jnp.broadcast_to(h_full[None, :, :], (ROWS, N, F))
        h_ii = jnp.broadcast_to(h_i[:, None, :], (ROWS, N, F))
        h_cat = jnp.concatenate([h_j, h_ii, x_norm], axis=-1)
        h_e_mtx = silu(silu(h_cat @ edge_w1 + edge_b1) @ edge_w2 + edge_b2)

        dist = (x_norm + INF * eye_i) * jnp.exp(log_gamma)
        euclid_att = jax.nn.softmax(-dist, axis=-2)
        sem = jax.nn.leaky_relu(h_e_mtx @ sem_w + sem_b, 0.2) - INF * eye_i
        sem_att = jax.nn.softmax(sem, axis=-2)
        comb_att = jax.nn.softmax(euclid_att * sem_att, axis=-2)

        h_e_att = (h_e_mtx[..., :, None] * comb_att[..., None, :]).reshape(ROWS, N, C)

        unit = x_minus_xt / (x_norm + EPS)
        comb_sum = jnp.einsum('ijc,ijd->icd', h_e_att, unit) / N
        comb_norm = (comb_sum ** 2).sum(-1)
        h_comb = silu(silu(comb_norm @ post_w1 + post_b1) @ post_w2 + post_b2)

        delta_v = jnp.einsum('icd,c->id', comb_sum, vmix_w)

        h_agg = h_e_att.sum(axis=-2)
        out = jnp.concatenate([h_i, h_agg, h_comb], axis=-1)
        out = silu(silu(out @ node_w1 + node_b1) @ node_w2 + node_b2)
        h_new = h_i + out

        v_scale = silu(h_new @ vel_w1 + vel_b1) @ vel_w2
        v_new = delta_v + v_scale * v_i
        x_new = x_i + v_new
        return h_new, x_new, v_new

    return shard_fn


def _run_shardmap(inputs):
    """Single compile: shard_map over an 8-device mesh, i-axis sharded."""
    import jax
    from jax.sharding import Mesh, PartitionSpec as P
    from jax.experimental.shard_map import shard_map

    h, x, v = (np.asarray(inputs[k]) for k in ('h', 'x', 'v'))
    wnames = ['edge_w1', 'edge_b1', 'edge_w2', 'edge_b2', 'sem_w', 'sem_b',
              'post_w1', 'post_b1', 'post_w2', 'post_b2', 'node_w1', 'node_b1',
              'node_w2', 'node_b2', 'vel_w1', 'vel_b1', 'vel_w2', 'vmix_w',
              'log_gamma']
    weights = [np.asarray(inputs[n]) for n in wnames]
    devices = jax.devices()[:N_CORES]
    assert len(devices) == N_CORES
    mesh = Mesh(np.asarray(devices), ('core',))
    eye = np.eye(N, dtype=np.float32)[:, :, None]

    # per-core args concatenated along axis 0 so each local shard is exactly
    # the per-core shape with no reshape inside the mapped body
    percore = {k: [] for k in ('h_full', 'x_full', 'h_i', 'x_i', 'v_i', 'eye_i')}
    for k in range(N_CORES):
        g0 = k * ROWS
        b, r0 = g0 // N, g0 % N
        percore['h_full'].append(h[b])
        percore['x_full'].append(x[b])
        percore['h_i'].append(h[b][r0:r0 + ROWS])
        percore['x_i'].append(x[b][r0:r0 + ROWS])
        percore['v_i'].append(v[b][r0:r0 + ROWS])
        percore['eye_i'].append(eye[r0:r0 + ROWS])
    cat = {k: np.concatenate(vs, axis=0) for k, vs in percore.items()}

    n_sharded = 6
    in_specs = (P('core'),) * n_sharded + (P(),) * len(weights)
    out_specs = (P('core'),) * 3
    fn = shard_map(_shard_fn_builder(), mesh=mesh, in_specs=in_specs,
                   out_specs=out_specs, check_rep=False)
    jfn = jax.jit(fn)
    outs = jfn(cat['h_full'], cat['x_full'], cat['h_i'], cat['x_i'],
               cat['v_i'], cat['eye_i'], *weights)
    h_new = np.asarray(outs[0]).reshape(B, N, F)
    x_new = np.asarray(outs[1]).reshape(B, N, 3)
    v_new = np.asarray(outs[2]).reshape(B, N, 3)
    return h_new, x_new, v_new


def _run_sharded(inputs):
    import jax
    jit_fn = jax.jit(_shard_fn_builder())

    h, x, v = inputs['h'], inputs['x'], inputs['v']
    wnames = ['edge_w1', 'edge_b1', 'edge_w2', 'edge_b2', 'sem_w', 'sem_b',
              'post_w1', 'post_b1', 'post_w2', 'post_b2', 'node_w1', 'node_b1',
              'node_w2', 'node_b2', 'vel_w1', 'vel_b1', 'vel_w2', 'vmix_w',
              'log_gamma']
    weights = [np.asarray(inputs[n]) for n in wnames]

    devices = jax.devices()
    if len(devices) < N_CORES:
        devices = devices * (N_CORES // max(1, len(devices)))
    eye = np.eye(N, dtype=np.float32)[:, :, None]  # [N, N, 1]

    futs = []
    for k in range(N_CORES):
        dev = devices[k % len(devices)]
        g0 = k * ROWS          # global flat row start
        b = g0 // N            # batch index of this shard
        r0 = g0 - b * N        # row offset within the batch
        args = [np.asarray(h[b]), np.asarray(x[b]),
                np.asarray(h[b][r0:r0 + ROWS]), np.asarray(x[b][r0:r0 + ROWS]),
                np.asarray(v[b][r0:r0 + ROWS]), eye[r0:r0 + ROWS]]
        args += weights
        dargs = [jax.device_put(a, dev) for a in args]
        futs.append(jit_fn(*dargs))

    # gather: concatenate the 8 shards back into [B, N, ...]
    h_parts, x_parts, v_parts = [], [], []
    for f in futs:
        hn, xn, vn = f
        h_parts.append(np.asarray(hn))
        x_parts.append(np.asarray(xn))
        v_parts.append(np.asarray(vn))
    h_new = np.concatenate(h_parts, 0).reshape(B, N, F)
    x_new = np.concatenate(x_parts, 0).reshape(B, N, 3)
    v_new = np.concatenate(v_parts, 0).reshape(B, N, 3)
    return h_new, x_new, v_new


def _run_numpy(inputs):
    """Pure-numpy fallback (exactly mirrors the reference math)."""
    h = np.asarray(inputs['h'], np.float32)
    x = np.asarray(inputs['x'], np.float32)
    v = np.asarray(inputs['v'], np.float32)
    g = {n: np.asarray(inputs[n], np.float32) for n in inputs if n not in ('h', 'x', 'v')}

    def silu(t):
        return t / (1.0 + np.exp(-t))

    def softmax(t, axis):
        t = t - t.max(axis=axis, keepdims=True)
        e = np.exp(t)
        return e / e.sum(axis=axis, keepdims=True)

    eye = np.eye(N, dtype=np.float32)[:, :, None]
    x_minus_xt = x[:, None, :, :] - x[:, :, None, :]
    x_norm = np.sqrt((x_minus_xt ** 2).sum(-1, keepdims=True) + EPS)
    h_j = np.broadcast_to(h[:, None, :, :], (B, N, N, F))
    h_i = np.broadcast_to(h[:, :, None, :], (B, N, N, F))
    h_cat = np.concatenate([h_j, h_i, np.asarray(x_norm)], axis=-1)
    h_e_mtx = silu(silu(h_cat @ g['edge_w1'] + g['edge_b1']) @ g['edge_w2'] + g['edge_b2'])

    dist = (x_norm + INF * eye) * np.exp(g['log_gamma'])
    euclid_att = softmax(-dist, axis=-2)
    sem_pre = h_e_mtx @ g['sem_w'] + g['sem_b']
    sem = np.where(sem_pre >= 0, sem_pre, 0.2 * sem_pre) - INF * eye
    sem_att = softmax(sem, axis=-2)
    comb_att = softmax(euclid_att * sem_att, axis=-2)

    h_e_att = (h_e_mtx[..., :, None] * comb_att[..., None, :]).reshape(B, N, N, C)
    unit = x_minus_xt / (x_norm + EPS)
    comb_sum = np.einsum('bijc,bijd->bicd', h_e_att, unit) / N
    comb_norm = (comb_sum ** 2).sum(-1)
    h_comb = silu(silu(comb_norm @ g['post_w1'] + g['post_b1']) @ g['post_w2'] + g['post_b2'])
    delta_v = np.einsum('bicd,c->bid', comb_sum, g['vmix_w'])
    h_agg = h_e_att.sum(axis=-2)
    out = np.concatenate([h, h_agg, h_comb], axis=-1)
    out = silu(silu(out @ g['node_w1'] + g['node_b1']) @ g['node_w2'] + g['node_b2'])
    h_new = h + out
    v_scale = silu(h_new @ g['vel_w1'] + g['vel_b1']) @ g['vel_w2']
    v_new = delta_v + v_scale * v
    x_new = x + v_new
    return h_new.astype(np.float32), x_new.astype(np.float32), v_new.astype(np.float32)


def kernel(**inputs):
    try:
        return _run_shardmap(inputs)
    except Exception:
        try:
            return _run_sharded(inputs)
        except Exception:
            return _run_numpy(inputs)


# revision 5
# speedup vs baseline: 139.0877x; 17.0881x over previous
"""DenseSAKELayer kernel — i-sharded (sequence-parallel) across 8 NeuronCores.

Contract: kernel(**inputs) takes the FULL unsharded inputs (h [2,256,64],
x [2,256,3], v [2,256,3], plus weights) and returns the full output tuple
(h_new, x_new, v_new).

Sharding: the (B*N)=512 query rows i are split into 8 contiguous shards of
64 rows (4 shards per batch element, so every shard lives in one batch).
All softmax / mean reductions are over j, so each shard only needs the
replicated per-batch x/h for the j axis — no cross-core communication.
Each shard is dispatched to its own NeuronCore; results are gathered and
concatenated on host.
"""
import numpy as np

B, N, F, H, HEADS = 2, 256, 64, 64, 4
C = HEADS * H
EPS = 1e-5
INF = 1e5
N_CORES = 8
ROWS = (B * N) // N_CORES  # 64 query rows per core


def _shard_fn_builder():
    import jax
    import jax.numpy as jnp

    def shard_fn(h_full, x_full, h_i, x_i, v_i, eye_i,
                 edge_w1, edge_b1, edge_w2, edge_b2, sem_w, sem_b,
                 post_w1, post_b1, post_w2, post_b2, node_w1, node_b1,
                 node_w2, node_b2, vel_w1, vel_b1, vel_w2, vmix_w, log_gamma):
        silu = jax.nn.silu
        # pairwise geometry: [rows, N, 3], entry [i, j] = x_j - x_i
        x_minus_xt = x_full[None, :, :] - x_i[:, None, :]
        x_norm = jnp.sqrt((x_minus_xt ** 2).sum(-1, keepdims=True) + EPS)

        h_j = jnp.broadcast_to(h_full[None, :, :], (ROWS, N, F))
        h_ii = jnp.broadcast_to(h_i[:, None, :], (ROWS, N, F))
        h_cat = jnp.concatenate([h_j, h_ii, x_norm], axis=-1)
        h_e_mtx = silu(silu(h_cat @ edge_w1 + edge_b1) @ edge_w2 + edge_b2)

        dist = (x_norm + INF * eye_i) * jnp.exp(log_gamma)
        euclid_att = jax.nn.softmax(-dist, axis=-2)
        sem = jax.nn.leaky_relu(h_e_mtx @ sem_w + sem_b, 0.2) - INF * eye_i
        sem_att = jax.nn.softmax(sem, axis=-2)
        comb_att = jax.nn.softmax(euclid_att * sem_att, axis=-2)

        h_e_att = (h_e_mtx[..., :, None] * comb_att[..., None, :]).reshape(ROWS, N, C)

        unit = x_minus_xt / (x_norm + EPS)
        comb_sum = jnp.einsum('ijc,ijd->icd', h_e_att, unit) / N
        comb_norm = (comb_sum ** 2).sum(-1)
        h_comb = silu(silu(comb_norm @ post_w1 + post_b1) @ post_w2 + post_b2)

        delta_v = jnp.einsum('icd,c->id', comb_sum, vmix_w)

        h_agg = h_e_att.sum(axis=-2)
        out = jnp.concatenate([h_i, h_agg, h_comb], axis=-1)
        out = silu(silu(out @ node_w1 + node_b1) @ node_w2 + node_b2)
        h_new = h_i + out

        v_scale = silu(h_new @ vel_w1 + vel_b1) @ vel_w2
        v_new = delta_v + v_scale * v_i
        x_new = x_i + v_new
        return h_new, x_new, v_new

    return shard_fn


_JFN_CACHE = {}


def _get_jfn():
    """Build the sharded executable once per process and reuse it."""
    import jax
    from jax.sharding import Mesh, PartitionSpec as P
    from jax.experimental.shard_map import shard_map
    if 'jfn' in _JFN_CACHE:
        return _JFN_CACHE['jfn']
    devices = jax.devices()[:N_CORES]
    assert len(devices) == N_CORES
    mesh = Mesh(np.asarray(devices), ('core',))
    n_weights = 19
    in_specs = (P('core'),) * 6 + (P(),) * n_weights
    out_specs = (P('core'),) * 3
    fn = shard_map(_shard_fn_builder(), mesh=mesh, in_specs=in_specs,
                   out_specs=out_specs, check_rep=False)
    jfn = jax.jit(fn)
    _JFN_CACHE['jfn'] = jfn
    return jfn


def _run_shardmap(inputs):
    """Single compile: shard_map over an 8-device mesh, i-axis sharded."""
    import jax
    from jax.sharding import Mesh, PartitionSpec as P
    from jax.experimental.shard_map import shard_map

    h, x, v = (np.asarray(inputs[k]) for k in ('h', 'x', 'v'))
    wnames = ['edge_w1', 'edge_b1', 'edge_w2', 'edge_b2', 'sem_w', 'sem_b',
              'post_w1', 'post_b1', 'post_w2', 'post_b2', 'node_w1', 'node_b1',
              'node_w2', 'node_b2', 'vel_w1', 'vel_b1', 'vel_w2', 'vmix_w',
              'log_gamma']
    weights = [np.asarray(inputs[n]) for n in wnames]
    devices = jax.devices()[:N_CORES]
    assert len(devices) == N_CORES
    mesh = Mesh(np.asarray(devices), ('core',))
    eye = np.eye(N, dtype=np.float32)[:, :, None]

    # per-core args concatenated along axis 0 so each local shard is exactly
    # the per-core shape with no reshape inside the mapped body
    percore = {k: [] for k in ('h_full', 'x_full', 'h_i', 'x_i', 'v_i', 'eye_i')}
    for k in range(N_CORES):
        g0 = k * ROWS
        b, r0 = g0 // N, g0 % N
        percore['h_full'].append(h[b])
        percore['x_full'].append(x[b])
        percore['h_i'].append(h[b][r0:r0 + ROWS])
        percore['x_i'].append(x[b][r0:r0 + ROWS])
        percore['v_i'].append(v[b][r0:r0 + ROWS])
        percore['eye_i'].append(eye[r0:r0 + ROWS])
    cat = {k: np.concatenate(vs, axis=0) for k, vs in percore.items()}

    jfn = _get_jfn()
    outs = jfn(cat['h_full'], cat['x_full'], cat['h_i'], cat['x_i'],
               cat['v_i'], cat['eye_i'], *weights)
    h_new = np.asarray(outs[0]).reshape(B, N, F)
    x_new = np.asarray(outs[1]).reshape(B, N, 3)
    v_new = np.asarray(outs[2]).reshape(B, N, 3)
    return h_new, x_new, v_new


def _run_sharded(inputs):
    import jax
    jit_fn = jax.jit(_shard_fn_builder())

    h, x, v = inputs['h'], inputs['x'], inputs['v']
    wnames = ['edge_w1', 'edge_b1', 'edge_w2', 'edge_b2', 'sem_w', 'sem_b',
              'post_w1', 'post_b1', 'post_w2', 'post_b2', 'node_w1', 'node_b1',
              'node_w2', 'node_b2', 'vel_w1', 'vel_b1', 'vel_w2', 'vmix_w',
              'log_gamma']
    weights = [np.asarray(inputs[n]) for n in wnames]

    devices = jax.devices()
    if len(devices) < N_CORES:
        devices = devices * (N_CORES // max(1, len(devices)))
    eye = np.eye(N, dtype=np.float32)[:, :, None]  # [N, N, 1]

    futs = []
    for k in range(N_CORES):
        dev = devices[k % len(devices)]
        g0 = k * ROWS          # global flat row start
        b = g0 // N            # batch index of this shard
        r0 = g0 - b * N        # row offset within the batch
        args = [np.asarray(h[b]), np.asarray(x[b]),
                np.asarray(h[b][r0:r0 + ROWS]), np.asarray(x[b][r0:r0 + ROWS]),
                np.asarray(v[b][r0:r0 + ROWS]), eye[r0:r0 + ROWS]]
        args += weights
        dargs = [jax.device_put(a, dev) for a in args]
        futs.append(jit_fn(*dargs))

    # gather: concatenate the 8 shards back into [B, N, ...]
    h_parts, x_parts, v_parts = [], [], []
    for f in futs:
        hn, xn, vn = f
        h_parts.append(np.asarray(hn))
        x_parts.append(np.asarray(xn))
        v_parts.append(np.asarray(vn))
    h_new = np.concatenate(h_parts, 0).reshape(B, N, F)
    x_new = np.concatenate(x_parts, 0).reshape(B, N, 3)
    v_new = np.concatenate(v_parts, 0).reshape(B, N, 3)
    return h_new, x_new, v_new


def _run_numpy(inputs):
    """Pure-numpy fallback (exactly mirrors the reference math)."""
    h = np.asarray(inputs['h'], np.float32)
    x = np.asarray(inputs['x'], np.float32)
    v = np.asarray(inputs['v'], np.float32)
    g = {n: np.asarray(inputs[n], np.float32) for n in inputs if n not in ('h', 'x', 'v')}

    def silu(t):
        return t / (1.0 + np.exp(-t))

    def softmax(t, axis):
        t = t - t.max(axis=axis, keepdims=True)
        e = np.exp(t)
        return e / e.sum(axis=axis, keepdims=True)

    eye = np.eye(N, dtype=np.float32)[:, :, None]
    x_minus_xt = x[:, None, :, :] - x[:, :, None, :]
    x_norm = np.sqrt((x_minus_xt ** 2).sum(-1, keepdims=True) + EPS)
    h_j = np.broadcast_to(h[:, None, :, :], (B, N, N, F))
    h_i = np.broadcast_to(h[:, :, None, :], (B, N, N, F))
    h_cat = np.concatenate([h_j, h_i, np.asarray(x_norm)], axis=-1)
    h_e_mtx = silu(silu(h_cat @ g['edge_w1'] + g['edge_b1']) @ g['edge_w2'] + g['edge_b2'])

    dist = (x_norm + INF * eye) * np.exp(g['log_gamma'])
    euclid_att = softmax(-dist, axis=-2)
    sem_pre = h_e_mtx @ g['sem_w'] + g['sem_b']
    sem = np.where(sem_pre >= 0, sem_pre, 0.2 * sem_pre) - INF * eye
    sem_att = softmax(sem, axis=-2)
    comb_att = softmax(euclid_att * sem_att, axis=-2)

    h_e_att = (h_e_mtx[..., :, None] * comb_att[..., None, :]).reshape(B, N, N, C)
    unit = x_minus_xt / (x_norm + EPS)
    comb_sum = np.einsum('bijc,bijd->bicd', h_e_att, unit) / N
    comb_norm = (comb_sum ** 2).sum(-1)
    h_comb = silu(silu(comb_norm @ g['post_w1'] + g['post_b1']) @ g['post_w2'] + g['post_b2'])
    delta_v = np.einsum('bicd,c->bid', comb_sum, g['vmix_w'])
    h_agg = h_e_att.sum(axis=-2)
    out = np.concatenate([h, h_agg, h_comb], axis=-1)
    out = silu(silu(out @ g['node_w1'] + g['node_b1']) @ g['node_w2'] + g['node_b2'])
    h_new = h + out
    v_scale = silu(h_new @ g['vel_w1'] + g['vel_b1']) @ g['vel_w2']
    v_new = delta_v + v_scale * v
    x_new = x + v_new
    return h_new.astype(np.float32), x_new.astype(np.float32), v_new.astype(np.float32)


def kernel(**inputs):
    try:
        return _run_shardmap(inputs)
    except Exception:
        try:
            return _run_sharded(inputs)
        except Exception:
            return _run_numpy(inputs)
